# revision 1
# baseline (speedup 1.0000x reference)
"""CRF loss kernel for Trainium2 (8 NeuronCores, data-parallel over batch).

Reference computation (see problem):
    score = einsum('blf,fk->blk', X, W);  forward/backward CRF messages over L;
    loss = mean_b(emit + trans - logZ).

Device algorithm (per core, batch shard of 1024):
  - score matmul done as: PE-transpose X tiles (bf16) -> Xt [F, b]; then
    matmul(lhsT=W_block[128,32], rhs=Xt) -> score [32-row group, b] in PSUM.
    4 batch-groups of 256 live at partition offsets 0/32/64/96 (26 labels +
    6 zero pad rows each).
  - expsc = exp(score - SHIFT) via ACT (fused PSUM->SBUF copy), bf16.
  - CRF forward recursion in probability domain:
      p_t = (BD^T @ p_{t-1}) * expsc_t,  BD = block-diag(exp(T)),
    renormalized every 2 steps by Z = group-sum of p (computed by a second
    matmul with a group-summing 0/1 matrix ZS), accumulating log Z via the
    ACT Ln accum_out. logZ_b = sum(log Z) + log(final sum) + L*SHIFT.
  - emit  = <A, W>,  A[f,k] = sum_j X[j,f]*onehot(y_j)[k]  (PE accumulation)
  - trans = <C, T>,  C[k,m] = sum_j onehot(y_j)[k]*onehot(y_{j+1})[m]
  - per-core output: [32*sum_b sum log Z, emit_total, trans_total, 0]
Host combines: loss = (sum_cores emit+trans - sumlog/32 - 1024*L*SHIFT)/8192.
"""

import numpy as np

B, L, F, K = 8192, 32, 128, 26
N_CORES = 8
BC = B // N_CORES            # batch per core
NCHUNK = BC // 128           # 8 chunks of 128 batch rows
GROUPS = 4                   # label-row groups packed on partitions
GB = BC // GROUPS            # 256 batch columns per group
SHIFT = 26.0

_cache = {}


def _make_consts():
    import ml_dtypes
    bf = ml_dtypes.bfloat16
    ident = np.eye(128, dtype=bf)
    iota = np.zeros((128, L * K), dtype=bf)
    for i in range(L):
        iota[:, i * K:(i + 1) * K] = np.arange(K, dtype=np.float32)[None, :]
    zs = np.zeros((128, 128), dtype=bf)
    for r in range(128):
        for c in range(128):
            if r // 32 == c // 32 and r % 32 < K:
                zs[r, c] = 1
    ones = np.ones((128, 1), dtype=np.float32)
    return ident, iota, zs, ones


def _build_program():
    import concourse.bass as bass  # noqa: F401
    import concourse.bacc as bacc
    import concourse.tile as tile
    from concourse import mybir
    from contextlib import ExitStack

    f32 = mybir.dt.float32
    bf16 = mybir.dt.bfloat16
    i32 = mybir.dt.int32
    AF = mybir.ActivationFunctionType
    ALU = mybir.AluOpType

    nc = bacc.Bacc("TRN2", target_bir_lowering=False)

    Xd = nc.dram_tensor("X", [BC, L, F], f32, kind="ExternalInput")
    Yd = nc.dram_tensor("Y", [BC, L], i32, kind="ExternalInput")
    IDENTd = nc.dram_tensor("IDENT", [128, 128], bf16, kind="ExternalInput")
    WBLKd = nc.dram_tensor("WBLK", [128, 32], bf16, kind="ExternalInput")
    BDd = nc.dram_tensor("BD", [128, 128], bf16, kind="ExternalInput")
    ZSd = nc.dram_tensor("ZS", [128, 128], bf16, kind="ExternalInput")
    IOTAd = nc.dram_tensor("IOTA", [128, L * K], bf16, kind="ExternalInput")
    WTd = nc.dram_tensor("WT", [K, 128], f32, kind="ExternalInput")
    T26d = nc.dram_tensor("T26", [K, K], f32, kind="ExternalInput")
    ONESd = nc.dram_tensor("ONES", [128, 1], f32, kind="ExternalInput")
    OUTd = nc.dram_tensor("out", [4, 1], f32, kind="ExternalOutput")

    with tile.TileContext(nc) as tc, ExitStack() as ctx:
        singles = ctx.enter_context(tc.tile_pool(name="singles", bufs=1))
        accp = ctx.enter_context(tc.tile_pool(name="accp", bufs=1, space="PSUM"))

        ident = singles.tile([128, 128], bf16)
        nc.sync.dma_start(out=ident, in_=IDENTd.ap())
        wblk = singles.tile([128, 32], bf16)
        nc.sync.dma_start(out=wblk, in_=WBLKd.ap())
        bd = singles.tile([128, 128], bf16)
        nc.sync.dma_start(out=bd, in_=BDd.ap())
        zsm = singles.tile([128, 128], bf16)
        nc.sync.dma_start(out=zsm, in_=ZSd.ap())
        iota = singles.tile([128, L * K], bf16)
        nc.sync.dma_start(out=iota, in_=IOTAd.ap())
        wt = singles.tile([K, 128], f32)
        nc.sync.dma_start(out=wt, in_=WTd.ap())
        t26 = singles.tile([K, K], f32)
        nc.sync.dma_start(out=t26, in_=T26d.ap())
        ones = singles.tile([128, 1], f32)
        nc.sync.dma_start(out=ones, in_=ONESd.ap())

        expsc = singles.tile([128, L * GB], bf16)      # [128, 8192]
        nshift = singles.tile([128, 1], f32)
        nc.vector.memset(nshift, -SHIFT)
        combo = singles.tile([128, 4], f32)
        nc.vector.memset(combo, 0.0)
        logacc = singles.tile([128, 16], f32)
        nc.vector.memset(logacc, 0.0)

        # A (emit) / C (trans) accumulators in separate PSUM banks.
        acc = accp.tile([K, 64], f32)
        accA = accp.tile([K, 128], f32, tag="accA")
        A_ps = accA[:, 0:128]
        C_ps = acc[:, 0:K]

        # ---------------- phase 1: scores, emit, trans ----------------
        with tc.tile_pool(name="xpool", bufs=2) as xpool, \
             tc.tile_pool(name="xtpool", bufs=2) as xtpool, \
             tc.tile_pool(name="ohpool", bufs=2) as ohpool, \
             tc.tile_pool(name="ypool", bufs=2) as ypool, \
             tc.tile_pool(name="trp", bufs=2, space="PSUM") as trp, \
             tc.tile_pool(name="scp", bufs=2, space="PSUM") as scp:
            for c in range(NCHUNK):
                g = c // 2
                coff = (c % 2) * 128
                xb = xpool.tile([128, L * F], bf16)
                nc.gpsimd.dma_start(
                    out=xb,
                    in_=Xd.ap()[c * 128:(c + 1) * 128].rearrange("b l f -> b (l f)"),
                )
                ysb = ypool.tile([128, L], i32, tag="ysb")
                nc.sync.dma_start(out=ysb, in_=Yd.ap()[c * 128:(c + 1) * 128])
                ybf = ypool.tile([128, L], bf16, tag="ybf")
                nc.vector.tensor_copy(out=ybf, in_=ysb)
                oh = ohpool.tile([128, L * K], bf16)
                nc.vector.tensor_tensor(
                    oh.rearrange("p (i k) -> p i k", k=K),
                    iota.rearrange("p (i k) -> p i k", k=K),
                    ybf.unsqueeze(2).to_broadcast([128, L, K]),
                    ALU.is_equal,
                )

                xt = xtpool.tile([128, L * F], bf16)
                for r in range(4):
                    tr = trp.tile([128, 1024], bf16)
                    for s in range(8):
                        i = r * 8 + s
                        nc.tensor.transpose(
                            tr[:, s * 128:(s + 1) * 128],
                            xb[:, i * 128:(i + 1) * 128],
                            ident,
                        )
                    nc.vector.tensor_copy(
                        out=xt[:, r * 1024:(r + 1) * 1024], in_=tr
                    )

                for r in range(4):
                    sc = scp.tile([128, 1024], f32)
                    for s in range(8):
                        i = r * 8 + s
                        nc.tensor.matmul(
                            sc[32 * g:32 * g + 32, s * 128:(s + 1) * 128],
                            lhsT=wblk,
                            rhs=xt[:, i * 128:(i + 1) * 128],
                            start=True, stop=True,
                            tile_position=(0, 32 * g),
                        )
                    dst = expsc.rearrange("p (t b) -> p t b", b=GB)[
                        32 * g:32 * g + 32, r * 8:(r + 1) * 8, coff:coff + 128
                    ]
                    src = sc.rearrange("p (s b) -> p s b", b=128)[32 * g:32 * g + 32]
                    nc.scalar.activation(
                        dst, src, AF.Exp,
                        bias=nshift[32 * g:32 * g + 32, 0:1],
                    )

                for i in range(L):
                    oh_i = oh[:, i * K:(i + 1) * K]
                    nc.tensor.matmul(
                        A_ps, lhsT=oh_i, rhs=xb[:, i * 128:(i + 1) * 128],
                        start=(c == 0 and i == 0),
                        stop=(c == NCHUNK - 1 and i == L - 1),
                        skip_group_check=True,
                    )
                    if i < L - 1:
                        nc.tensor.matmul(
                            C_ps, lhsT=oh_i, rhs=oh[:, (i + 1) * K:(i + 2) * K],
                            start=(c == 0 and i == 0),
                            stop=(c == NCHUNK - 1 and i == L - 2),
                            skip_group_check=True,
                        )

        # emit/trans reduction
        with tc.tile_pool(name="fin", bufs=1) as fin:
            ae = fin.tile([K, 128], f32)
            nc.vector.tensor_tensor(ae, A_ps, wt, ALU.mult)
            nc.vector.tensor_reduce(
                combo[0:K, 1:2], ae, axis=mybir.AxisListType.X, op=ALU.add
            )
            ce = fin.tile([K, K], f32)
            nc.vector.tensor_tensor(ce, C_ps, t26, ALU.mult)
            nc.vector.tensor_reduce(
                combo[0:K, 2:3], ce, axis=mybir.AxisListType.X, op=ALU.add
            )

        # ---------------- phase 2: CRF recursion ----------------
        with tc.tile_pool(name="pp", bufs=2) as pp, \
             tc.tile_pool(name="vp", bufs=2) as vp, \
             tc.tile_pool(name="rzp", bufs=2) as rzp, \
             tc.tile_pool(name="lnp", bufs=2) as lnp, \
             tc.tile_pool(name="up", bufs=2, space="PSUM") as up, \
             tc.tile_pool(name="zp", bufs=2, space="PSUM") as zp:
            p_prev = expsc[:, 0:GB]
            nidx = 0
            for t in range(1, L):
                u = up.tile([128, GB], f32)
                nc.tensor.matmul(u, lhsT=bd, rhs=p_prev, start=True, stop=True)
                e_sl = expsc[:, t * GB:(t + 1) * GB]
                if t % 2 == 0:
                    v = vp.tile([128, GB], bf16)
                    nc.vector.tensor_mul(v, u, e_sl)
                    z = zp.tile([128, GB], f32)
                    nc.tensor.matmul(z, lhsT=zsm, rhs=v, start=True, stop=True)
                    rz = rzp.tile([128, GB], f32)
                    nc.vector.reciprocal(rz, z)
                    lnscr = lnp.tile([128, GB], bf16)
                    nc.scalar.activation(
                        lnscr, z, AF.Ln, accum_out=logacc[:, nidx:nidx + 1]
                    )
                    nidx += 1
                    pn = pp.tile([128, GB], bf16)
                    nc.vector.tensor_mul(pn, v, rz)
                else:
                    pn = pp.tile([128, GB], bf16)
                    nc.vector.tensor_mul(pn, u, e_sl)
                p_prev = pn
            zf = zp.tile([128, GB], f32)
            nc.tensor.matmul(zf, lhsT=zsm, rhs=p_prev, start=True, stop=True)
            lnscr = lnp.tile([128, GB], bf16)
            nc.scalar.activation(
                lnscr, zf, AF.Ln, accum_out=logacc[:, nidx:nidx + 1]
            )
            nidx += 1

            nc.vector.tensor_reduce(
                combo[:, 0:1], logacc, axis=mybir.AxisListType.X, op=ALU.add
            )
            res_ps = acc[0:4, 40:41]
            nc.tensor.matmul(res_ps, lhsT=combo, rhs=ones, start=True, stop=True)
            outsb = singles.tile([4, 1], f32)
            nc.vector.tensor_copy(out=outsb, in_=res_ps)
            nc.sync.dma_start(out=OUTd.ap(), in_=outsb)

    nc.compile()
    return nc


def _get_program():
    if "nc" not in _cache:
        _cache["nc"] = _build_program()
    return _cache["nc"]


def _make_in_maps(X, y, W, T):
    import ml_dtypes
    bf = ml_dtypes.bfloat16
    ident, iota, zs, ones = _make_consts()
    Wb = W.astype(bf)
    wblk = np.zeros((128, 32), dtype=bf)
    wblk[:, :K] = Wb
    expT = np.exp(T.astype(np.float64)).astype(bf)
    bdm = np.zeros((128, 128), dtype=bf)
    for g in range(GROUPS):
        bdm[32 * g:32 * g + K, 32 * g:32 * g + K] = expT
    wtm = W.T.astype(np.float32).copy()
    t26 = T.astype(np.float32).copy()

    in_maps = []
    for cidx in range(N_CORES):
        Xc = np.ascontiguousarray(X[cidx * BC:(cidx + 1) * BC]).astype(np.float32)
        Yc = np.ascontiguousarray(y[cidx * BC:(cidx + 1) * BC]).astype(np.int32)
        in_maps.append({
            "X": Xc, "Y": Yc,
            "IDENT": ident, "WBLK": wblk, "BD": bdm, "ZS": zs,
            "IOTA": iota, "WT": wtm, "T26": t26, "ONES": ones,
        })
    return in_maps


def _combine(results):
    total = 0.0
    for r in results:
        o = np.asarray(r["out"], dtype=np.float64)
        sumlog = o[0, 0] / 32.0
        emit = o[1, 0]
        trans = o[2, 0]
        total += emit + trans - sumlog - BC * L * SHIFT
    return np.float32(total / B)


def kernel(X, y, W, T):
    from concourse.bass_utils import run_bass_kernel_spmd
    nc = _get_program()
    in_maps = _make_in_maps(np.asarray(X), np.asarray(y),
                            np.asarray(W), np.asarray(T))
    res = run_bass_kernel_spmd(nc, in_maps, list(range(N_CORES)))
    return _combine(res.results)

